# revision 5
# baseline (speedup 1.0000x reference)
"""CRF negative-log-likelihood on 8 Trainium2 NeuronCores (Bass/Tile).

Sharding: data-parallel over batch — each of the 8 cores runs the forward
algorithm for its 32 sequences; the tiny numerator (label gathers) and the
final reduction are done on host.

Device algorithm (per core), derived in linear domain to make each scan step
one small matmul + one elementwise multiply:

  alpha_t = logsumexp-recurrence  ==>  p_t[j,b] = exp(alpha_t[j,b] - shift)
  p_t = u'_t  *  (E^T @ p_{t-1}),   E = exp(trans),
  u'_t[j,b] = exp(logits[b,t,j] - c_t)   (c_t: host-chosen per-step shift
                                          keeping p in f32 range)

The log-partition for sequence b is needed only at t = seq_len_b - 1:
  log_norm_b = log(colsum_j p_t[j,b]) + (accumulated rescales) + cumsum(c)_t
Colsums are computed on demand by a ones-column matmul (M=1) and stored to a
z-buffer; every 32 steps a colsum is measured and p is multiplied by its
reciprocal 16 steps later (lagged renorm, off the critical path), with the
measured value logged so the host can reconstruct log_norm exactly.

The kernel is specialized at build time to the union (over all 8 cores, so
one SPMD program serves every core) of harvest time-steps from seq_lens.
"""

import numpy as np

B, T, L = 256, 1024, 64
NCORES = 8
SEQ_PER_CORE = B // NCORES          # 32
CHAINS = 2                          # independent scan chains per core
S = SEQ_PER_CORE // CHAINS          # 16 sequences per chain tile
RENORM = 32                         # measure period
LAG = 16                            # apply lag
UCHUNK = 64                         # t-steps per exp/DMA chunk

_cache = {}


def _build_schedule(seq_lens):
    """Per-chain sorted event t's (harvest union over cores + measures)."""
    measures = [t for t in range(T) if t % RENORM == RENORM - 1 and t + LAG < T]
    chains = []
    for ch in range(CHAINS):
        hs = set()
        for c in range(NCORES):
            for k in range(S):
                b = c * SEQ_PER_CORE + ch * S + k
                hs.add(int(seq_lens[b]) - 1)
        events = sorted(hs | set(measures))
        chains.append({
            "events": events,
            "col": {t: i for i, t in enumerate(events)},
            "measures": measures,
        })
    return chains


def _build_program(sched):
    import concourse.bass as bass
    import concourse.bacc as bacc
    import concourse.mybir as mybir
    from concourse import tile

    f32 = mybir.dt.float32
    nc = bacc.Bacc("TRN2", target_bir_lowering=False, debug=False)

    lg = nc.dram_tensor("lg", [L, T * SEQ_PER_CORE], f32, kind="ExternalInput").ap()
    eaug = nc.dram_tensor("eaug", [L, L], f32, kind="ExternalInput").ap()
    ones_col = nc.dram_tensor("ones_col", [L, 1], f32, kind="ExternalInput").ap()
    ones_row = nc.dram_tensor("ones_row", [1, L], f32, kind="ExternalInput").ap()
    ncols = sum(len(c["events"]) for c in sched) * S
    zout = nc.dram_tensor("zout", [1, ncols], f32, kind="ExternalOutput").ap()

    zbase = [0, len(sched[0]["events"]) * S]  # per-chain column base in zout

    with tile.TileContext(nc, trace_sim=False) as tc:
        with (
            tc.tile_pool(name="persist", bufs=1) as persist,
            tc.tile_pool(name="stage", bufs=3) as stage,
            tc.tile_pool(name="psum", bufs=1, space="PSUM") as pp,
        ):
            e_sb = persist.tile([L, L], f32, name="e_sb")
            nc.sync.dma_start(e_sb[:], eaug[:])
            onec_sb = persist.tile([L, 1], f32, name="onec_sb")
            nc.sync.dma_start(onec_sb[:], ones_col[:])
            oner_sb = persist.tile([1, L], f32, name="oner_sb")
            nc.sync.dma_start(oner_sb[:], ones_row[:])

            zbuf = persist.tile([1, ncols], f32, name="zbuf")

            # streamed u' production: DMA logits chunk -> ACT Exp -> u tile
            nchunks = T // UCHUNK
            u_tiles = []
            for c in range(nchunks):
                st = stage.tile([L, UCHUNK * SEQ_PER_CORE], f32, name=f"st{c}",
                                tag="stage")
                nc.sync.dma_start(
                    st[:],
                    lg[:, c * UCHUNK * SEQ_PER_CORE:(c + 1) * UCHUNK * SEQ_PER_CORE])
                ut = persist.tile([L, UCHUNK * SEQ_PER_CORE], f32, name=f"u{c}")
                nc.scalar.activation(ut[:], st[:],
                                     mybir.ActivationFunctionType.Exp)
                u_tiles.append(ut)

            def u_slice(t, ch):
                cidx, toff = divmod(t, UCHUNK)
                c0 = toff * SEQ_PER_CORE + ch * S
                return u_tiles[cidx][:, c0:c0 + S]

            p_t = [persist.tile([L, S], f32, name=f"p{ch}") for ch in range(CHAINS)]
            rinv = [persist.tile([1, S], f32, name=f"rinv{ch}")
                    for ch in range(CHAINS)]

            # rb tiles: renorm broadcast factors, alive measure -> apply
            pending_rb = [None] * CHAINS

            for ch in range(CHAINS):
                nc.vector.tensor_copy(p_t[ch][:], u_slice(0, ch))

            applies = {m + LAG: m for m in sched[0]["measures"]}

            for t in range(T):
                for ch in range(CHAINS):
                    sc = sched[ch]
                    if t > 0:
                        s = pp.tile([L, S], f32, name=f"s{ch}_{t}",
                                    tag=f"s{ch}", bufs=2)
                        nc.tensor.matmul(s[:], e_sb[:], p_t[ch][:],
                                         start=True, stop=True)
                        nc.vector.tensor_mul(p_t[ch][:], s[:], u_slice(t, ch))
                    if t in applies:
                        rb = pending_rb[ch]
                        assert rb is not None
                        nc.vector.tensor_mul(p_t[ch][:], p_t[ch][:], rb[:])
                        pending_rb[ch] = None
                    if t in sc["col"]:
                        z = pp.tile([1, S], f32, name=f"z{ch}_{t}",
                                    tag="z", bufs=2)
                        nc.tensor.matmul(z[:], onec_sb[:], p_t[ch][:],
                                         start=True, stop=True)
                        col = zbase[ch] + sc["col"][t] * S
                        nc.vector.tensor_copy(zbuf[:, col:col + S], z[:])
                        if t in applies.values() and t + LAG < T:
                            nc.vector.reciprocal(rinv[ch][:], z[:])
                            rb = pp.tile([L, S], f32, name=f"rb{ch}_{t}",
                                         tag="rb", bufs=2)
                            nc.tensor.matmul(rb[:], oner_sb[:], rinv[ch][:],
                                             start=True, stop=True)
                            pending_rb[ch] = rb

            nc.sync.dma_start(zout[:], zbuf[:])

    nc.compile()
    return nc


def kernel(logits, labels, seq_lens, trans):
    from concourse.bass_utils import run_bass_kernel_spmd

    logits = np.asarray(logits, dtype=np.float32)
    labels64 = np.asarray(labels).astype(np.int64)
    seq_lens64 = np.asarray(seq_lens).astype(np.int64)
    trans = np.asarray(trans, dtype=np.float32)

    # ---- host prep ----
    E = np.exp(trans)
    c = logits.mean(axis=(0, 2)).astype(np.float64)
    c[1:] += np.log(E.sum(axis=0).mean())
    cumc = np.cumsum(c)

    sched = _build_schedule(seq_lens64)
    key = tuple(tuple(s["events"]) for s in sched)
    if key not in _cache:
        _cache.clear()
        _cache[key] = _build_program(sched)
    nc = _cache[key]

    shifted = (logits - c[None, :, None].astype(np.float32))
    in_maps = []
    for core in range(NCORES):
        lgc = shifted[core * SEQ_PER_CORE:(core + 1) * SEQ_PER_CORE]  # [32,T,L]
        lgc = np.ascontiguousarray(lgc.transpose(2, 1, 0).reshape(L, T * SEQ_PER_CORE))
        in_maps.append({
            "lg": lgc,
            "eaug": np.ascontiguousarray(E),
            "ones_col": np.ones((L, 1), dtype=np.float32),
            "ones_row": np.ones((1, L), dtype=np.float32),
        })

    res = run_bass_kernel_spmd(nc, in_maps, core_ids=list(range(NCORES)))
    _cache["last_run"] = (nc, in_maps)

    # ---- host reconstruction of log_norm ----
    zbase = [0, len(sched[0]["events"]) * S]
    lnb = np.zeros(B, dtype=np.float64)
    for core in range(NCORES):
        zv = res.results[core]["zout"][0].astype(np.float64)
        for ch in range(CHAINS):
            sc = sched[ch]
            for k in range(S):
                b = core * SEQ_PER_CORE + ch * S + k
                th = int(seq_lens64[b]) - 1
                zh = zv[zbase[ch] + sc["col"][th] * S + k]
                acc = np.log(zh) + cumc[th]
                for m in sc["measures"]:
                    if m + LAG <= th:
                        acc += np.log(zv[zbase[ch] + sc["col"][m] * S + k])
                lnb[b] = acc

    # ---- numerator on host ----
    pos = np.arange(T)
    mask = pos[None, :] < seq_lens64[:, None]
    unary = np.take_along_axis(
        logits.astype(np.float64), labels64[..., None], axis=2)[..., 0]
    unary_score = np.where(mask, unary, 0.0).sum(axis=1)
    pair = trans.astype(np.float64)[labels64[:, :-1], labels64[:, 1:]]
    pair_mask = pos[None, 1:] < seq_lens64[:, None]
    binary_score = np.where(pair_mask, pair, 0.0).sum(axis=1)
    seq_score = unary_score + binary_score

    nll = np.sum(lnb - seq_score)
    return np.array(nll, dtype=np.float32)


def profile_hw_ns():
    """Re-run the last compiled program with NTFF tracing; returns exec ns."""
    import tempfile
    from concourse.bass_utils import run_bass_kernel_spmd

    if "last_run" not in _cache:
        return None
    nc, in_maps = _cache["last_run"]
    tmpdir = tempfile.mkdtemp(prefix="crf_trace_")
    res = run_bass_kernel_spmd(nc, in_maps, core_ids=list(range(NCORES)),
                               trace=True, tmpdir=tmpdir)
    _cache["last_trace"] = (tmpdir, res.instructions_and_trace,
                            res.profile_json)
    print(f"trace dir: {tmpdir}")
    if res.instructions_and_trace:
        print(f"trace path: {res.instructions_and_trace[1]}")
    return res.exec_time_ns
